# revision 10
# baseline (speedup 1.0000x reference)
# Trainium2 Bass kernel for nn_CustomImageCosineSimLoss (N=4096, D=512, 8 cores).
#
# Strategy (sharding_hint): shard image rows across the 8 cores (data parallel
# over i); text features / instruction ids replicated. Each core computes its
# [512, 4096] block of both pairwise matrices and 16 relu partial sums plus
# per-row min-max stats; the host combines the partials with two closed-form
# corrections and divides by N^2 (the "all-reduce").
#
# Math per core (L=512 local rows):
#   device part = sum_ij relu(cos_ij - w_ij)        (over ALL pairs)
# with sim'_ij = that_i . t_j  (= sim_ij / n_i, so the min-max weights
# w_ij = (sim'_ij - mn'_i) * invr'_i match the reference up to an O(1e-7)
# epsilon shift) and cos_ij = ihat_i . that_j.  The host adds the exact
# aligned-pair term sum_aligned (1 - cos) (fp64 group sums, O(N*D)) and
# subtracts its own estimate of the aligned relu terms the device included,
# using the device-exported stats (mirroring the bf16 rounding of invr').
#
# Engine mapping per [128, 1024] wide tile: PE does the sim'/cos matmuls in
# fp8 DoubleRow mode (fp32 PSUM) and folds the -sim'*invr' term into the cos
# PSUM via a diag(-invr') bf16 matmul (diagonal built on DVE as
# identity * ninvr); ACT computes relu(pc + mn'*invr') straight off PSUM
# with per-row sum accumulation and does most sim' PSUM->SBUF copies (one
# per i-tile goes to DVE for balance).  DVE computes the min/max per half
# row via a tensor_tensor tree (0.79 ns/elem vs tensor_reduce's 1.1) plus
# the scalar chain.  Inputs arrive as a few large flat tensors (fewer DMA
# descriptors), all triggered on the sync queue's hardware DGE.
import numpy as np
import ml_dtypes

import concourse.mybir as mybir
import concourse.tile as tile
from concourse import bacc
from concourse.bass import ts

BF16 = mybir.dt.bfloat16
F32 = mybir.dt.float32
FP8 = mybir.dt.float8e4
AF = mybir.ActivationFunctionType
OP = mybir.AluOpType
PM = mybir.MatmulPerfMode
nf8 = ml_dtypes.float8_e4m3
nbf = ml_dtypes.bfloat16

N, D, G, NCORES = 4096, 512, 64, 8
L = N // NCORES            # 512 local rows per core
KT = D // 128              # 4 contraction subtiles
KP = KT // 2               # 2 DoubleRow pairs
IT = L // 128              # 4 local i-tiles
JT = N // 512              # 8 j-blocks
WT = JT // 2               # 4 wide (1024-col) tiles per i-tile
EPS_W = 1e-6

_CACHE = {}


def _build_program():
    nc = bacc.Bacc("TRN2", target_bir_lowering=False, debug=False,
                   enable_asserts=True, num_devices=NCORES)

    d_txt_a = nc.dram_tensor("txt_a", [128, KT * 2048], FP8,
                             kind="ExternalInput").ap()
    d_txt_b = nc.dram_tensor("txt_b", [128, KT * 2048], FP8,
                             kind="ExternalInput").ap()
    d_that_all = nc.dram_tensor("that_all", [128, KT * N], FP8,
                                kind="ExternalInput").ap()
    d_that_loc = nc.dram_tensor("that_loc", [128, KT * L], FP8,
                                kind="ExternalInput").ap()
    d_ihat_loc = nc.dram_tensor("ihat_loc", [128, KT * L], FP8,
                                kind="ExternalInput").ap()
    d_ident = nc.dram_tensor("ident", [128, 128], BF16,
                             kind="ExternalInput").ap()
    d_partials = nc.dram_tensor("partials", [128, IT * WT], F32,
                                kind="ExternalOutput").ap()
    d_stats = nc.dram_tensor("stats_out", [128, 2 * IT], F32,
                             kind="ExternalOutput").ap()

    with tile.TileContext(nc) as tc:
        with (
            tc.tile_pool(name="persist", bufs=1) as pp,
            tc.tile_pool(name="sims", bufs=3) as psim,
            tc.tile_pool(name="tree", bufs=2) as ptree,
            tc.tile_pool(name="diags", bufs=3) as pdg,
            tc.tile_pool(name="junk", bufs=2) as pj,
            tc.tile_pool(name="stats", bufs=2) as pst,
            tc.tile_pool(name="psA", bufs=2, space="PSUM") as ppsA,
            tc.tile_pool(name="psB", bufs=2, space="PSUM") as ppsB,
        ):
            # loads, in need order, all on the sync queue's hardware DGE
            that_loc = pp.tile([128, KT * L], FP8)
            nc.sync.dma_start(that_loc[:], d_that_loc)
            that_loc_v = that_loc[:].rearrange("p (c i) -> p c i", c=KT)

            txt_a = pp.tile([128, KT * 2048], FP8)
            nc.sync.dma_start(txt_a[:], d_txt_a)
            txt_a_v = txt_a[:].rearrange("p (c j) -> p c j", c=KT)
            txt_b = pp.tile([128, KT * 2048], FP8)
            nc.sync.dma_start(txt_b[:], d_txt_b)
            txt_b_v = txt_b[:].rearrange("p (c j) -> p c j", c=KT)

            def txtj(jt):
                v = txt_a_v if jt < 4 else txt_b_v
                return v[:, :, ts(jt % 4, 512)]

            ihat_loc = pp.tile([128, KT * L], FP8)
            nc.sync.dma_start(ihat_loc[:], d_ihat_loc)
            ihat_loc_v = ihat_loc[:].rearrange("p (c i) -> p c i", c=KT)
            ident = pp.tile([128, 128], BF16)
            nc.sync.dma_start(ident[:], d_ident)

            that_all = pp.tile([128, KT * N], FP8)
            nc.sync.dma_start(that_all[:], d_that_all)
            that_all_v = that_all[:].rearrange("p (c j) -> p c j", c=KT)

            comb = pp.tile([128, IT * WT], F32)
            stats_sb = pp.tile([128, 2 * IT], F32)   # invr / mninvr per it
            sims, halves, stats = {}, {}, {}

            def emit_minmax_half(sim_sb, hh, mnH, mxH):
                # tt-tree min & max over sim_sb[:, hh*2048:(hh+1)*2048]
                base = sim_sb[:, ts(hh, 2048)]
                sc = ptree.tile([128, 2048], BF16, tag="tree")
                for op, dst in ((OP.min, mnH), (OP.max, mxH)):
                    nc.vector.tensor_tensor(out=sc[:, 0:1024],
                                            in0=base[:, 0:1024],
                                            in1=base[:, 1024:2048], op=op)
                    nc.vector.tensor_tensor(out=sc[:, 1024:1536],
                                            in0=sc[:, 0:512],
                                            in1=sc[:, 512:1024], op=op)
                    nc.vector.tensor_tensor(out=sc[:, 0:256],
                                            in0=sc[:, 1024:1280],
                                            in1=sc[:, 1280:1536], op=op)
                    nc.vector.tensor_tensor(out=sc[:, 256:384],
                                            in0=sc[:, 0:128],
                                            in1=sc[:, 128:256], op=op)
                    nc.vector.tensor_reduce(out=dst[:, hh:hh + 1],
                                            in_=sc[:, 256:384],
                                            axis=mybir.AxisListType.X, op=op)

            def emit_sim(it):
                sim_sb = psim.tile([128, N], BF16, tag="sim")
                mnH = pst.tile([128, 2], F32, tag="mnH")
                mxH = pst.tile([128, 2], F32, tag="mxH")
                for w in range(WT):
                    ps = ppsA.tile([128, 1024], F32, tag="mmA")
                    for h in range(2):
                        jt = 2 * w + h
                        for kp in range(KP):
                            nc.tensor.matmul(
                                ps[:, ts(h, 512)],
                                that_loc_v[:, 2 * kp:2 * kp + 2, ts(it, 128)],
                                txtj(jt)[:, 2 * kp:2 * kp + 2, :],
                                start=(kp == 0), stop=(kp == KP - 1),
                                perf_mode=PM.DoubleRow)
                    if w == 2:
                        # one wide copy per i-tile on DVE for engine balance
                        nc.vector.tensor_scalar_add(
                            out=sim_sb[:, ts(w, 1024)], in0=ps[:], scalar1=0.0)
                    else:
                        nc.scalar.copy(sim_sb[:, ts(w, 1024)], ps[:])
                    if w % 2 == 1:
                        emit_minmax_half(sim_sb, w // 2, mnH, mxH)
                sims[it] = sim_sb
                halves[it] = (mnH, mxH)

            def emit_stats(it):
                mnH, mxH = halves[it]
                mn = pst.tile([128, 1], F32, tag="mn")
                nc.vector.tensor_reduce(out=mn[:], in_=mnH[:],
                                        axis=mybir.AxisListType.X, op=OP.min)
                mx = pst.tile([128, 1], F32, tag="mx")
                nc.vector.tensor_reduce(out=mx[:], in_=mxH[:],
                                        axis=mybir.AxisListType.X, op=OP.max)
                rng = pst.tile([128, 1], F32, tag="rng")
                nc.vector.tensor_tensor(out=rng[:], in0=mx[:], in1=mn[:],
                                        op=OP.subtract)
                nc.vector.tensor_scalar_add(out=rng[:], in0=rng[:], scalar1=EPS_W)
                invr = stats_sb[:, 2 * it:2 * it + 1]
                nc.vector.reciprocal(invr, rng[:])
                ninvr = pst.tile([128, 1], F32, tag="ninvr")
                nc.vector.tensor_scalar_mul(out=ninvr[:], in0=invr, scalar1=-1.0)
                mninvr = stats_sb[:, 2 * it + 1:2 * it + 2]
                nc.vector.tensor_tensor(out=mninvr, in0=mn[:], in1=invr,
                                        op=OP.mult)
                # diag(-invr') in bf16: identity rows scaled per partition
                diag = pdg.tile([128, 128], BF16, tag="diag")
                nc.vector.tensor_scalar_mul(out=diag[:], in0=ident[:],
                                            scalar1=ninvr[:])
                stats[it] = (diag, mninvr)

            def emit_cos(it):
                sim_sb = sims[it]
                diag, mninvr = stats[it]
                for w in range(WT):
                    pc = ppsB.tile([128, 1024], F32, tag="mmB")
                    for h in range(2):
                        jt = 2 * w + h
                        for kp in range(KP):
                            nc.tensor.matmul(
                                pc[:, ts(h, 512)],
                                ihat_loc_v[:, 2 * kp:2 * kp + 2, ts(it, 128)],
                                that_all_v[:, 2 * kp:2 * kp + 2, ts(jt, 512)],
                                start=(kp == 0), stop=False,
                                perf_mode=PM.DoubleRow)
                        nc.tensor.matmul(pc[:, ts(h, 512)], diag[:],
                                         sim_sb[:, ts(jt, 512)],
                                         start=False, stop=True)
                    # relu(pc + mn'*invr') with per-row sum accumulation
                    jk = pj.tile([128, 1024], BF16, tag="junk")
                    nc.scalar.activation(
                        out=jk[:], in_=pc[:], func=AF.Relu, bias=mninvr,
                        scale=1.0,
                        accum_out=comb[:, it * WT + w:it * WT + w + 1])

            # software pipeline (PE order): sim0 sim1 sim2 cos0 sim3 cos1 cos2 cos3
            emit_sim(0)
            emit_stats(0)
            emit_sim(1)
            emit_stats(1)
            emit_sim(2)
            emit_stats(2)
            emit_cos(0)
            emit_sim(3)
            emit_stats(3)
            emit_cos(1)
            emit_cos(2)
            emit_cos(3)

            nc.sync.dma_start(d_partials, comb[:])
            nc.sync.dma_start(d_stats, stats_sb[:])

    nc.compile()
    return nc


def _flat_dmajor(arr_T8, cols):
    # [D, cols] d-major -> flat SBUF layout [128, KT*cols]
    return np.ascontiguousarray(
        arr_T8.reshape(KT, 128, cols).transpose(1, 0, 2).reshape(128, KT * cols))


def _host_prep(image_features, text_features, instr_d):
    img = np.asarray(image_features, np.float64)
    txt = np.asarray(text_features, np.float64)
    ins = np.asarray(instr_d).astype(np.int64)

    nt = np.linalg.norm(txt, axis=1)
    ni = np.linalg.norm(img, axis=1)
    that = txt / nt[:, None]
    ihat = img / ni[:, None]

    txt_T8 = np.ascontiguousarray(txt.T.astype(np.float32)).astype(nf8)
    that_T8 = np.ascontiguousarray(that.T.astype(np.float32)).astype(nf8)

    shared = {
        "txt_a": _flat_dmajor(np.ascontiguousarray(txt_T8[:, :2048]), 2048),
        "txt_b": _flat_dmajor(np.ascontiguousarray(txt_T8[:, 2048:]), 2048),
        "that_all": _flat_dmajor(that_T8, N),
        "ident": np.eye(128, dtype=nbf),
    }

    in_maps = []
    for c in range(NCORES):
        sl = slice(c * L, (c + 1) * L)
        m = dict(shared)
        m["that_loc"] = _flat_dmajor(np.ascontiguousarray(that_T8[:, sl]), L)
        m["ihat_loc"] = _flat_dmajor(
            np.ascontiguousarray(ihat[sl].T.astype(np.float32)).astype(nf8), L)
        in_maps.append(m)

    # exact aligned-pair contribution sum_aligned (1 - cos), fp64 on host
    cnt = np.bincount(ins, minlength=G).astype(np.float64)
    IH = np.zeros((G, D))
    np.add.at(IH, ins, ihat)
    TH = np.zeros((G, D))
    np.add.at(TH, ins, that)
    corr = float((cnt ** 2).sum() - (IH * TH).sum())
    return in_maps, corr, ins, txt, that, ihat


def _aligned_relu_sub(res, ins, txt, that, ihat):
    # Reconstruct per-row invr / mn*invr from the device stats dumps
    # (mirroring the bf16 rounding the diag matmul applied to invr), then
    # estimate the aligned-pair relu terms the device summed (to subtract).
    invr = np.zeros(N)
    mninvr = np.zeros(N)
    for c, r in enumerate(res.results):
        st = np.asarray(r["stats_out"], np.float64)     # [128, 2*IT]
        for it in range(IT):
            rows = slice(c * L + it * 128, c * L + it * 128 + 128)
            invr[rows] = st[:, 2 * it].astype(np.float32).astype(nbf)
            mninvr[rows] = st[:, 2 * it + 1]
    sub = 0.0
    for g in range(G):
        idx = np.where(ins == g)[0]
        if idx.size == 0:
            continue
        cosg = ihat[idx] @ that[idx].T
        simg = that[idx] @ txt[idx].T
        arg = cosg - simg * invr[idx][:, None] + mninvr[idx][:, None]
        sub += np.maximum(arg, 0.0).sum()
    return sub


def kernel(**inputs) -> np.ndarray:
    from concourse.bass_utils import run_bass_kernel_spmd

    if "nc" not in _CACHE:
        _CACHE["nc"] = _build_program()
    nc = _CACHE["nc"]
    in_maps, corr, ins, txt, that, ihat = _host_prep(**inputs)
    res = run_bass_kernel_spmd(nc, in_maps, core_ids=list(range(NCORES)),
                               trace=False)
    _CACHE["last_results"] = res
    total = np.float64(corr)
    for r in res.results:
        total += np.asarray(r["partials"], np.float64).sum()
    total -= _aligned_relu_sub(res, ins, txt, that, ihat)
    return np.float32(total / (N * N))
